# Initial kernel scaffold
#
"""Stochastic-LIF neuron kernel for Trainium2 (8 NeuronCores).

Reference recurrence per element (b, n), over T=128 time steps:
    u_t = 0.5 * u_{t-1} + x_t
    o_t = (u_t > 1)
    u_t = u_t * (1 - o_t)        # hard reset to 0 on spike

Strategy:
  - Shard batch dim B=32 across 8 cores (4 per core). Per core the
    elements form a [128 partitions, 256 free] tile (4 b x 8192 n).
  - State kept as v (pre-reset potential). One fused custom DVE op per
    time step: v' = 0.5 * select(v <= 1, v, 0) + x_t   (~1 elem/cycle).
  - Spike output o = sign(v' - 1) on the ACT engine, saturating
    float->uint8 conversion maps {-1,0,1} -> {0,0,1} = (v' > 1).
  - x streamed in / o streamed out in CHUNK_T-step chunks, u8 output
    (4x less DMA); host converts to float32.
"""

import os

import numpy as np

B, T, N = 32, 128, 8192
NCORES = 8
BPC = B // NCORES          # batches per core
P = 128                    # SBUF partitions
F = BPC * N // P           # free dim per step = 256
PPB = P // BPC             # partition rows per batch = 32
CHUNK_T = 8                # time steps per DMA chunk

_cache = {}


def _register_custom_op():
    import concourse.dve_ops as dve_ops

    if "LIF_STEP_ANT" in dve_ops._SUB_OPCODE_FOR_NAME:
        return dve_ops._SUB_OPCODE_FOR_NAME and next(
            op for op in dve_ops.OPS if op.name == "LIF_STEP_ANT"
        )

    from concourse.dve_spec import C0, C1, Spec, Src0, Src1, Zero, select

    def _ref(in0, in1, s0, s1, imm2):
        u = np.where(in0 <= s1, in0, 0.0).astype(np.float32)
        return (u * s0 + in1).astype(np.float32)

    op = dve_ops.DveOp(
        "LIF_STEP_ANT",
        Spec(body=select(Src0 <= C1, Src0, Zero) * C0 + Src1, reference=_ref),
        subdim=False,
        uops_sha={"v3": "73713d2c766d7eeb", "v4": "f73a18201e32e28c"},
    )
    dve_ops.OPS.append(op)
    dve_ops.CUSTOM_DVE_SPECS[op.name] = op.spec
    dve_ops._SUB_OPCODE_FOR_NAME[op.name] = (
        dve_ops._CUSTOM_DVE_ROW_BASE + len(dve_ops.OPS) - 1
    )
    return op


def _build_nc():
    import concourse.bacc as bacc
    import concourse.mybir as mybir
    from concourse.tile import TileContext

    lif_op = _register_custom_op()

    nc = bacc.Bacc()
    f32 = mybir.dt.float32
    u8 = mybir.dt.uint8

    x_d = nc.dram_tensor("x", [BPC, T, N], f32, kind="ExternalInput")
    o_d = nc.dram_tensor("o", [BPC, T, N], u8, kind="ExternalOutput")

    # [b, t, (p f)] -> per-b views [p, t, f] for DMA (p = n // F)
    x_v = x_d[:].rearrange("b t (p f) -> b p t f", f=F)
    o_v = o_d[:].rearrange("b t (p f) -> b p t f", f=F)

    nchunks = T // CHUNK_T
    with TileContext(nc) as tc:
        with (
            tc.tile_pool(name="xin", bufs=3) as xpool,
            tc.tile_pool(name="oout", bufs=3) as opool,
            tc.tile_pool(name="state", bufs=2) as vpool,
        ):
            v = vpool.tile([P, F], f32, tag="v")
            nc.vector.memset(v[:], 0.0)
            for c in range(nchunks):
                t0 = c * CHUNK_T
                xt = xpool.tile([P, CHUNK_T * F], f32, tag="x")
                xt3 = xt[:].rearrange("p (t f) -> p t f", f=F)
                ot = opool.tile([P, CHUNK_T * F], u8, tag="o")
                ot3 = ot[:].rearrange("p (t f) -> p t f", f=F)
                for b in range(BPC):
                    nc.sync.dma_start(
                        out=xt3[b * PPB : (b + 1) * PPB],
                        in_=x_v[b, :, t0 : t0 + CHUNK_T],
                    )
                for j in range(CHUNK_T):
                    vn = vpool.tile([P, F], f32, tag="v")
                    nc.vector._custom_dve(
                        lif_op,
                        out=vn[:],
                        in0=v[:],
                        in1=xt3[:, j],
                        s0=0.5,
                        s1=1.0,
                    )
                    # o = sign(v - 1) in {-1,0,1}; f32->u8 saturates -> (v>1)
                    nc.scalar.activation(
                        ot3[:, j],
                        vn[:],
                        mybir.ActivationFunctionType.Sign,
                        bias=-1.0,
                        scale=1.0,
                    )
                    v = vn
                for b in range(BPC):
                    nc.sync.dma_start(
                        out=o_v[b, :, t0 : t0 + CHUNK_T],
                        in_=ot3[b * PPB : (b + 1) * PPB],
                    )
    return nc


def _get_nc():
    if "nc" not in _cache:
        _cache["nc"] = _build_nc()
    return _cache["nc"]


def kernel(x):
    from concourse.bass_utils import run_bass_kernel_spmd

    nc = _get_nc()
    x = np.ascontiguousarray(np.asarray(x, dtype=np.float32))
    xs = x.reshape(NCORES, BPC, T, N)
    in_maps = [{"x": xs[i]} for i in range(NCORES)]
    res = run_bass_kernel_spmd(
        nc,
        in_maps,
        core_ids=list(range(NCORES)),
        trace=bool(int(os.environ.get("LIF_TRACE", "0"))),
    )
    if res.exec_time_ns is not None:
        print(f"HW exec time: {res.exec_time_ns} ns")
        _cache["exec_time_ns"] = res.exec_time_ns
        _cache["trace"] = res.instructions_and_trace
    o = np.stack([res.results[i]["o"] for i in range(NCORES)])
    return o.reshape(B, T, N).astype(np.float32)


# revision 7
# speedup vs baseline: 2.5071x; 2.5071x over previous
"""Stochastic-LIF neuron kernel for Trainium2 (8 NeuronCores).

Reference recurrence per element (b, n), over T=128 time steps:
    u_t = 0.5 * u_{t-1} + x_t
    o_t = (u_t > 1)
    u_t = u_t * (1 - o_t)        # hard reset to 0 on spike

Strategy:
  - Shard batch dim B=32 across 8 cores (4 per core). Per core the
    elements form a [128 partitions, 256 free] tile (4 b x 8192 n).
  - State kept as v (pre-reset potential). One fused custom DVE op per
    time step: v' = 0.5 * select(v <= 1, v, 0) + x_t   (~1 elem/cycle).
  - Spike output o = sign(v' - 1) on the ACT engine, saturating
    float->uint8 conversion maps {-1,0,1} -> {0,0,1} = (v' > 1).
  - x streamed in / o streamed out in CHUNK_T-step chunks, u8 output
    (4x less DMA); host converts to float32.
"""

import os

import numpy as np

B, T, N = 32, 128, 8192
NCORES = 8
BPC = B // NCORES          # batches per core
P = 128                    # SBUF partitions
F = BPC * N // P           # free dim per step = 256
PPB = P // BPC             # partition rows per batch = 32
CHUNK_T = 8                # time steps per DMA chunk

_cache = {}


def _register_custom_op():
    import concourse.dve_ops as dve_ops

    if "LIF_STEP_ANT" in dve_ops._SUB_OPCODE_FOR_NAME:
        return dve_ops._SUB_OPCODE_FOR_NAME and next(
            op for op in dve_ops.OPS if op.name == "LIF_STEP_ANT"
        )

    from concourse.dve_spec import C0, C1, Spec, Src0, Src1, Zero, select

    def _ref(in0, in1, s0, s1, imm2):
        u = np.where(in0 <= s1, in0, 0.0).astype(np.float32)
        return (u * s0 + in1).astype(np.float32)

    op = dve_ops.DveOp(
        "LIF_STEP_ANT",
        Spec(body=select(Src0 <= C1, Src0, Zero) * C0 + Src1, reference=_ref),
        subdim=False,
        uops_sha={"v3": "73713d2c766d7eeb", "v4": "f73a18201e32e28c"},
    )
    dve_ops.OPS.append(op)
    dve_ops.CUSTOM_DVE_SPECS[op.name] = op.spec
    dve_ops._SUB_OPCODE_FOR_NAME[op.name] = (
        dve_ops._CUSTOM_DVE_ROW_BASE + len(dve_ops.OPS) - 1
    )
    return op


def _build_nc(repeat=1):
    import concourse.bacc as bacc
    import concourse.mybir as mybir
    from concourse.tile import TileContext

    lif_op = _register_custom_op()

    nc = bacc.Bacc()
    f32 = mybir.dt.float32
    u8 = mybir.dt.uint8

    x_d = nc.dram_tensor("x", [BPC, T, N], f32, kind="ExternalInput")
    o_d = nc.dram_tensor("o", [BPC, T, N], u8, kind="ExternalOutput")

    # [b, t, (p f)] -> per-b views [p, t, f] for DMA (p = n // F)
    x_v = x_d[:].rearrange("b t (p f) -> b p t f", f=F)
    o_v = o_d[:].rearrange("b t (p f) -> b p t f", f=F)

    nchunks = T // CHUNK_T
    with TileContext(nc) as tc:
        with (
            tc.tile_pool(name="xin", bufs=3) as xpool,
            tc.tile_pool(name="oout", bufs=3) as opool,
            tc.tile_pool(name="state", bufs=2) as vpool,
            tc.tile_pool(name="consts", bufs=1) as cpool,
        ):
            bias_m1 = cpool.tile([P, 1], f32, tag="bias")
            nc.vector.memset(bias_m1[:], -1.0)
            for _rep in range(repeat):
                v = vpool.tile([P, F], f32, tag="v")
                nc.vector.memset(v[:], 0.0)
                for c in range(nchunks):
                    t0 = c * CHUNK_T
                    xt = xpool.tile([P, CHUNK_T * F], f32, tag="x")
                    xt3 = xt[:].rearrange("p (t f) -> p t f", f=F)
                    ot = opool.tile([P, CHUNK_T * F], u8, tag="o")
                    ot3 = ot[:].rearrange("p (t f) -> p t f", f=F)
                    for b in range(BPC):
                        nc.sync.dma_start(
                            out=xt3[b * PPB : (b + 1) * PPB],
                            in_=x_v[b, :, t0 : t0 + CHUNK_T],
                        )
                    for j in range(CHUNK_T):
                        vn = vpool.tile([P, F], f32, tag="v")
                        nc.vector._custom_dve(
                            lif_op,
                            out=vn[:],
                            in0=v[:],
                            in1=xt3[:, j],
                            s0=0.5,
                            s1=1.0,
                        )
                        # o = sign(v-1) in {-1,0,1}; f32->u8 saturates -> (v>1)
                        nc.scalar.activation(
                            ot3[:, j],
                            vn[:],
                            mybir.ActivationFunctionType.Sign,
                            bias=bias_m1[:],
                            scale=1.0,
                        )
                        v = vn
                    for b in range(BPC):
                        nc.sync.dma_start(
                            out=o_v[b, :, t0 : t0 + CHUNK_T],
                            in_=ot3[b * PPB : (b + 1) * PPB],
                        )
    nc.compile()
    return nc


def _get_nc():
    if "nc" not in _cache:
        _cache["nc"] = _build_nc()
    return _cache["nc"]


def kernel(x):
    from concourse.bass_utils import run_bass_kernel_spmd

    nc = _get_nc()
    x = np.ascontiguousarray(np.asarray(x, dtype=np.float32))
    xs = x.reshape(NCORES, BPC, T, N)
    in_maps = [{"x": xs[i]} for i in range(NCORES)]
    res = run_bass_kernel_spmd(
        nc,
        in_maps,
        core_ids=list(range(NCORES)),
        trace=bool(int(os.environ.get("LIF_TRACE", "0"))),
    )
    if res.exec_time_ns is not None:
        print(f"HW exec time: {res.exec_time_ns} ns")
        _cache["exec_time_ns"] = res.exec_time_ns
        _cache["trace"] = res.instructions_and_trace
    o = np.stack([res.results[i]["o"] for i in range(NCORES)])
    return o.reshape(B, T, N).astype(np.float32)
